# revision 26
# baseline (speedup 1.0000x reference)
"""Causal self-attention (B=2, S=2048, HID=1024, 16 heads x 64) on 8 trn2
NeuronCores.

Sharding: data-parallel over batch (cores 0-3 -> batch 0, cores 4-7 ->
batch 1), tensor-parallel over heads (4 heads per core via Wqk/Wv column
slices). Each core computes its 4 heads end-to-end; the [S, S] score
matrix stays core-local.

Per-core layout choices:
  - All matmul operands are bf16 (inputs are cast host-side); PSUM
    accumulation stays fp32.
  - q, k are produced TRANSPOSED ([head_cols, S]) so score matmuls need
    no on-device transposes; scores are computed transposed ([sk, sq])
    so the P @ v matmul consumes exp(scores) directly from SBUF.
  - v carries an appended ones-column per head; the attention output
    matmul then yields softmax row-sums in an extra partition row for
    free (no max-subtraction is needed: scores are O(5) so exp is safe
    in fp32, and masked entries are zeroed multiplicatively post-exp
    with a GPSIMD multiply against a [128,128] triangle mask input).
  - EVERY stationary operand is a 4B-aligned, 128-column bf16 slice so
    the compiler emits FWL (fast weight load) LDWEIGHTS: FWL loads go
    to the background weight buffer and hide behind the running matmul
    stream; non-FWL loads serialize (~130ns each, measured). For P @ v
    this means v head slots are 66 cols (64 + ones + pad) and the lhsT
    is the OVERLAPPING 128-col window starting at each head slot - the
    extra columns produce garbage PSUM rows 66-127 that are never read.
  - v bias is a free DVE add (bias pre-broadcast host-side) fused into
    the PSUM->SBUF copy, replacing a K=1 matmul on the PE.
  - Heads are processed in pairs: the two K=64 score matmuls sit in PE
    row-groups 0-63 / 64-127 and run concurrently in the array.
  - The P @ v matmuls run 2 chunks BEHIND the score/exp stream, so the
    in-order PE queue never stalls on the ACT engine's exp latency.
  - Inputs arrive host-prepacked in the exact SBUF image as large
    contiguous DMAs split across the two HWDGE issue queues (Sync +
    Scalar), ordered so the first projection unit's inputs (wq + both
    xq0 halves) land first on separate queues.
  - The output is stored in the SBUF-native layout ([128, a, c, 256],
    one fully contiguous 512KB DMA per stripe) and re-laid-out on the
    host; the previous row-major store scattered 512B segments and
    cost ~13us of post-compute DMA tail.
  - Stripes run 0,1,3,2 so the trailing region still has projection
    work as PE filler; pair-tail P @ v + PSUM drain + head finalization
    (transpose + normalize) are deferred into the following units via a
    priority queue.
"""
import sys

for _p in ("/opt/trn_rl_repo",):
    if _p not in sys.path:
        sys.path.insert(0, _p)

import numpy as np

B, S, HID = 2, 2048, 1024
NH, HD = 16, 64
NHL = 4            # heads per core
WC = NHL * HD      # 256 local q/k weight cols
HS = HD + 2        # 66-col head slot in v: 64 + ones col + pad (4B align)
VC = NHL * HS      # 264 local v cols
VPAD = 3 * HS + 128  # 326: last head's 128-col lhsT window end
VT = 328           # padded v tile width (even)
NT = S // 128      # 16 key chunks
NA = S // 512      # 4 query stripes
NK = HID // 128    # 8 contraction chunks
LAG = 3            # P @ v trails the score/exp stream by this many chunks
NWARM = 10         # scratch warmup matmuls bridging the input DMA front

_NC = None


def _build():
    from concourse import bacc, mybir
    from concourse.tile import TileContext

    FP = mybir.dt.float32
    BF = mybir.dt.bfloat16
    Exp = mybir.ActivationFunctionType.Exp

    nc = bacc.Bacc("TRN2", target_bir_lowering=False, debug=False, num_devices=8)

    # all inputs are host-prepacked into the exact SBUF image, so every
    # input DMA is a fully-contiguous identity copy
    xq_d = [nc.dram_tensor(f"xq{q}", [128, NK * 512], BF, kind="ExternalInput")
            for q in range(4)]
    wq = nc.dram_tensor("wq", [128, NK * WC], BF, kind="ExternalInput")
    wk = nc.dram_tensor("wk", [128, NK * WC], BF, kind="ExternalInput")
    wv = nc.dram_tensor("wv", [128, NK * VC], BF, kind="ExternalInput")
    wvl_d = nc.dram_tensor("wvl", [128, VT], BF, kind="ExternalInput")
    bqk = nc.dram_tensor("bqk", [128, 4], FP, kind="ExternalInput")
    tri_d = nc.dram_tensor("tri", [128, 128], BF, kind="ExternalInput")
    # output leaves as the raw UNNORMALIZED [65, 512] P@v blocks (64 head
    # dims + softmax-denominator row) in bf16: the host does the divide +
    # transpose (trivial numpy). This removes all 64 PE transposes, the
    # DVE recip/normalize chain, and the identity input; bf16 adds ~0.2%
    # rounding against a 2e-2 rel-err budget.
    out = nc.dram_tensor("out", [HD + 1, NA * 2 * 1024], BF,
                         kind="ExternalOutput")

    with TileContext(nc) as tc:
        with (
            tc.tile_pool(name="inp", bufs=1) as inp,
            tc.tile_pool(name="ptp", bufs=8) as ptp,
            tc.tile_pool(name="osb", bufs=4) as osb,
            tc.tile_pool(name="G", bufs=3, space="PSUM") as gp,
            tc.tile_pool(name="oT", bufs=2, space="PSUM") as otp,
        ):
            # PE warmup on a scratch tile (no DMA dependency, so it starts
            # right after the engine preambles): keeps the PE busy through
            # the HAM SHORT window while the input DMA streams, so the
            # projection stream starts at 2.4 GHz instead of 1.2. memset
            # on gpsimd - its preamble finishes ~1us before the DVE's.
            scratch = inp.tile([128, 512], BF, name="scratch")
            nc.gpsimd.memset(scratch[:, :], 0.0)
            # preload the GPSIMD ucode IRAM (~6us, hidden in the preamble)
            # so the first in-stream gpsimd mask-multiply doesn't pay it
            gsc = inp.tile([128, 1], BF, name="gsc")
            nc.gpsimd.memset(gsc[:, :], 0.0)
            warm = gp.tile([128, 1024], mybir.dt.float32, tag="G", name="warm")
            for _ in range(NWARM):
                nc.tensor.matmul(warm[:, :512], lhsT=scratch[:, :128],
                                 rhs=scratch[:, :], start=True, stop=True)

            # ---- persistent inputs in SBUF, contiguous identity DMAs
            # split across the two HWDGE issue queues (Sync + Scalar).
            # The first projection unit needs wq + BOTH xq0 halves; wk
            # follows ~2us later, wv ~2us after that. Each queue moves
            # ~180 GB/s, so pair the critical tensors across queues.
            # critical-path order: wq/xq0_0 first on their queues (first
            # q half-unit), then wk / xq0_1 (k half + q second half), then
            # wv (v units, consumed ~20us in), tri (first boundary mask,
            # ~21us), xq1 (fillers from uidx 4), ident (first finish_head,
            # ~26us), and the remaining x quarters.
            xq = [[None, None] for _ in range(4)]
            # wq and the first xq0 half are themselves DMA'd in halves:
            # the queues ramp slowly (~70GB/s early), and the Tile dep
            # tracker is region-exact, so the first projection matmuls
            # start as soon as the first 256KB of each lands (~11us)
            wq_sb = inp.tile([128, NK * WC], BF, name="wq")
            nc.sync.dma_start(wq_sb[:, :4 * WC], wq[:, :4 * WC])
            xq[0][0] = inp.tile([128, 4 * 512], BF, name="xq0_0")
            nc.scalar.dma_start(xq[0][0][:, :1024], xq_d[0][:, 0:1024])
            nc.sync.dma_start(wq_sb[:, 4 * WC:], wq[:, 4 * WC:])
            nc.scalar.dma_start(xq[0][0][:, 1024:], xq_d[0][:, 1024:2048])
            wk_sb = inp.tile([128, NK * WC], BF, name="wk")
            nc.sync.dma_start(wk_sb[:, :], wk[:, :])
            xq[0][1] = inp.tile([128, 4 * 512], BF, name="xq0_1")
            nc.scalar.dma_start(xq[0][1][:, :], xq_d[0][:, 2048:4096])
            bqk_sb = inp.tile([128, 4], FP, name="bqk")
            nc.sync.dma_start(bqk_sb[:, :], bqk[:, :])
            wv_sb = inp.tile([128, NK * VC], BF, name="wv")
            nc.scalar.dma_start(wv_sb[:, :], wv[:, :])
            tri = inp.tile([128, 128], BF, name="tri")
            nc.scalar.dma_start(tri[:, :], tri_d[:, :])
            wvl_sb = inp.tile([128, VT], BF, name="wvl")
            nc.scalar.dma_start(wvl_sb[:, :], wvl_d[:, :])
            xq[1][0] = inp.tile([128, NK * 512], BF, name="xq1")
            nc.sync.dma_start(xq[1][0][:, :], xq_d[1][:, :])
            for qtr, eng in ((2, nc.scalar), (3, nc.sync)):
                t = inp.tile([128, NK * 512], BF, name=f"xq{qtr}")
                eng.dma_start(t[:, :], xq_d[qtr][:, :])
                xq[qtr][0] = t

            def xk(k, qtr):
                if qtr == 0:
                    t = xq[0][k // 4]
                    return t[:, (k % 4) * 512:(k % 4 + 1) * 512]
                return xq[qtr][0][:, k * 512:(k + 1) * 512]

            # split by S-quarter so interleaved later-quarter projection
            # writes can't false-depend against earlier attention reads
            qT_sb = [[inp.tile([128, 512], BF, name=f"qT{t}_{n}")
                      for n in range(4)] for t in range(2)]
            kT_sb = [[inp.tile([128, 512], BF, name=f"kT{t}_{n}")
                      for n in range(4)] for t in range(2)]
            v_sb = [inp.tile([128, VT], BF, name=f"v{c}") for c in range(NT)]

            # ---- projection emitters ----
            def proj_qk_unit(wt, bcol, dst, t, qtr):
                g = gp.tile([128, 1024], mybir.dt.float32, tag="G", name="g")
                for k in range(NK):
                    nc.tensor.matmul(
                        g[:, :512],
                        lhsT=wt[:, k * WC + t * 128:k * WC + (t + 1) * 128],
                        rhs=xk(k, qtr),
                        start=(k == 0), stop=(k == NK - 1),
                    )
                nc.vector.tensor_scalar_add(
                    dst[t][qtr][:, :], g[:, :512], bqk_sb[:, bcol + t:bcol + t + 1]
                )

            def proj_v_unit(c):
                qtr, cc = divmod(c, 4)
                g = gp.tile([128, 1024], mybir.dt.float32, tag="G", name="g")
                for k in range(NK):
                    nc.tensor.matmul(
                        g[:, :VC],
                        lhsT=xk(k, qtr)[:, cc * 128:(cc + 1) * 128],
                        rhs=wv_sb[:, k * VC:(k + 1) * VC],
                        start=(k == 0), stop=(k == NK - 1),
                    )
                # bias (+ ones col) via a free DVE add fused into the
                # PSUM->SBUF copy; cols VC..VT copy stale psum garbage to
                # keep the overlapping 128-col P@v weight windows in
                # written territory (their rows 66-127 are never read)
                nc.vector.tensor_add(v_sb[c][:, :], g[:, :VT], wvl_sb[:, :])

            # ---- attention emitters ----
            # score/exp for ONE key chunk b of a head PAIR:
            # g = [h0-slice | h1-slice], one exp covers both heads
            def score_exp_unit(a, ht, b):
                g = gp.tile([128, 1024], mybir.dt.float32, tag="G", name="g")
                kn, ko = divmod(b * 128, 512)
                # diagonal chunks: columns < off are fully masked -- skip
                # them in the score matmul, the exp, and the P @ v matmul
                off = max(0, (b - 4 * a) * 128)
                for hh in range(2):
                    hb = hh * 64
                    nc.tensor.matmul(
                        g[:, hh * 512 + off:(hh + 1) * 512],
                        lhsT=kT_sb[ht][kn][hb:hb + 64, ko:ko + 128],
                        rhs=qT_sb[ht][a][hb:hb + 64, off:],
                        start=True, stop=True,
                    )
                pt = ptp.tile([128, 1024], BF, tag="pt", name="pt")
                if off:
                    gv = g[:, :].rearrange("p (h w) -> p h w", h=2)[:, :, off:]
                    pv = pt[:, :].rearrange("p (h w) -> p h w", h=2)[:, :, off:]
                    nc.scalar.activation(pv, gv, Exp, scale=HD ** -0.5)
                else:
                    nc.scalar.activation(pt[:, :], g[:, :], Exp, scale=HD ** -0.5)
                if b >= 4 * a:
                    # triangular boundary block: multiplicative mask. On
                    # GPSIMD (otherwise idle) to keep the DVE queue short --
                    # DVE reads are what free PSUM slots for the PE.
                    for hh in range(2):
                        c0 = hh * 512 + off
                        nc.gpsimd.tensor_mul(
                            pt[:, c0:c0 + 128], pt[:, c0:c0 + 128], tri[:, :]
                        )
                return pt

            def av_unit(a, ht, b, nchunks, oTs, pt):
                off = max(0, (b - 4 * a) * 128)
                for hh in range(2):
                    h = 2 * ht + hh
                    # lhsT is the 128-col window starting at head h's v
                    # slot: cols 0-63 v, 64 ones, 65 pad, 66-127 the next
                    # heads' data -> out rows 66-127 garbage, never read.
                    # 128 cols + 4B-aligned base => FWL, load is hidden.
                    nc.tensor.matmul(
                        oTs[hh][:, off:],
                        lhsT=v_sb[b][:, h * HS:h * HS + 128],
                        rhs=pt[:, hh * 512 + off:(hh + 1) * 512],
                        start=(b == 0), stop=(b == nchunks - 1),
                    )

            # ---- phase 1: the minimum needed by stripe a=0 head pair 0.
            # q/k t=0 units are emitted in k-HALVES: the first 4 matmuls
            # need only wq + the first 512KB half of xq0 (per-queue DMA
            # moves ~130GB/s, so halving the critical bytes starts the
            # real stream ~4us earlier); the second halves then wait on
            # xq0's other half / wk, which land while the first run.
            def proj_qk_half(wt, bcol, dst, t, ks, g):
                for k in ks:
                    nc.tensor.matmul(
                        g[:, :512],
                        lhsT=wt[:, k * WC + t * 128:k * WC + (t + 1) * 128],
                        rhs=xk(k, 0),
                        start=(k == 0), stop=(k == NK - 1),
                    )
                if ks[-1] == NK - 1:
                    nc.vector.tensor_scalar_add(
                        dst[t][0][:, :], g[:, :512], bqk_sb[:, bcol + t:bcol + t + 1]
                    )

            gq = gp.tile([128, 1024], mybir.dt.float32, tag="G", name="g")
            gk = gp.tile([128, 1024], mybir.dt.float32, tag="G", name="g")
            proj_qk_half(wq_sb, 0, qT_sb, 0, [0, 1, 2, 3], gq)
            proj_qk_half(wk_sb, 2, kT_sb, 0, [0, 1, 2, 3], gk)
            proj_qk_half(wq_sb, 0, qT_sb, 0, [4, 5, 6, 7], gq)
            proj_qk_half(wk_sb, 2, kT_sb, 0, [4, 5, 6, 7], gk)
            # v units 0/1 are NOT emitted here: the first scores go first,
            # so the exp stream starts ~2us earlier and recycles the G
            # psum ring for the v/proj fillers (ring depth is 3; a filler
            # allocating before the first exp completes stalls the PE)

            # remaining projection units are doled out between attention
            # units, scheduled (just) before their first consumer, keeping
            # the PE busy while ACT works through the exp stream
            def q_(t, qtr):
                return lambda: proj_qk_unit(wq_sb, 0, qT_sb, t, qtr)

            def k_(t, qtr):
                return lambda: proj_qk_unit(wk_sb, 2, kT_sb, t, qtr)

            def v_(c):
                return lambda: proj_v_unit(c)

            # placement: just-before-first-consumer deadlines, spread so
            # every region keeps the PE slightly ahead of the exp stream.
            # Stripes run 0,1,3,2: the trailing stripe-2 region (24 units)
            # then still has its own q/k projections left as PE filler,
            # where stripe 3 last would leave the PE starved (and the HAM
            # clock gate re-throttling) for its final 16 units.
            # NOTE: stripe 3 consumes ALL kT quarters and v chunks, so only
            # the stripe-2 q projections can be held back for the tail
            filler = {
                1: [v_(0)], 2: [v_(1), q_(1, 0)], 3: [v_(2), k_(1, 0)],
                4: [v_(3)], 5: [q_(0, 1)], 6: [k_(0, 1)], 7: [v_(4)],
                8: [v_(5)], 10: [v_(6)], 12: [v_(7)], 13: [q_(1, 1)],
                15: [k_(1, 1)], 17: [q_(0, 3)], 19: [k_(0, 3)],
                21: [v_(8)], 23: [k_(0, 2)],
                25: [v_(9)], 26: [v_(10)], 28: [v_(11)], 30: [v_(12)],
                32: [v_(13)], 34: [v_(14)], 36: [v_(15)], 38: [q_(1, 3)],
                39: [k_(1, 3)], 44: [k_(1, 2)], 50: [q_(0, 2)],
                58: [q_(1, 2)],
            }

            deferred = []          # drain/store closures fed into the stream

            # ---- phases 2+3: attention, software-pipelined ----
            uidx = 0
            for a in (0, 1, 3, 2):
                nchunks = 4 * a + 4
                for ht in range(2):
                    oTs = [otp.tile([128, 512], mybir.dt.float32,
                                    tag="oT", name="oT") for _ in range(2)]
                    pend = []
                    for b in range(nchunks):
                        pend.append((b, score_exp_unit(a, ht, b)))
                        if len(pend) > LAG:
                            bb, pt = pend.pop(0)
                            av_unit(a, ht, bb, nchunks, oTs, pt)
                        for f in filler.get(uidx, ()):
                            f()
                        for _ in range(2 if len(deferred) > 6 else 1):
                            if deferred:
                                deferred.pop(0)()
                        uidx += 1
                    # the tail P @ v matmuls and the oT psum drain flow into
                    # the NEXT pair's units (via the priority end of the
                    # deferred queue): by then the exp stream has caught up,
                    # so the in-order PE queue never waits at pair boundaries
                    bb0, pt0 = pend.pop(0)
                    av_unit(a, ht, bb0, nchunks, oTs, pt0)

                    def tail_av(a_=a, ht_=ht, p_=tuple(pend), n_=nchunks,
                                o_=oTs):
                        for bb, pt in p_:
                            av_unit(a_, ht_, bb, n_, o_, pt)

                    def drain(a_=a, ht_=ht, o_=oTs):
                        # pack both heads' raw [65, 512] blocks side by side
                        # and ship them as ONE contiguous 2KB-per-partition
                        # DMA; the host divides by the denominator row and
                        # transposes
                        ot = osb.tile([HD + 1, 1024], BF, tag="oTsb",
                                      name="oTsb")
                        for hh in range(2):
                            nc.vector.tensor_copy(
                                ot[:, hh * 512:(hh + 1) * 512],
                                o_[hh][:HD + 1, :])

                        def store(a_=a_, ht_=ht_, t_=ot):
                            nc.sync.dma_start(
                                out[:, (a_ * 2 + ht_) * 1024:
                                    (a_ * 2 + ht_ + 1) * 1024],
                                t_[:, :])
                        deferred.append(store)

                    deferred.insert(0, drain)
                    deferred.insert(0, tail_av)
            while deferred:
                deferred.pop(0)()

    nc.compile()
    return nc


def _get_nc():
    global _NC
    if _NC is None:
        _NC = _build()
    return _NC


def make_in_maps(hidden_states, Wqk, bqk, Wv, bv):
    from ml_dtypes import bfloat16

    x = np.asarray(hidden_states, dtype=np.float32)
    Wqk = np.asarray(Wqk, dtype=np.float32)
    bqk = np.asarray(bqk, dtype=np.float32)
    Wv = np.asarray(Wv, dtype=np.float32)
    bv = np.asarray(bv, dtype=np.float32)

    def pack(w):
        # [1024, C] -> SBUF image [128, 8*C] (k-chunk-major columns)
        c = w.shape[1]
        return np.ascontiguousarray(
            w.reshape(NK, 128, c).transpose(1, 0, 2).reshape(128, NK * c)
        ).astype(bfloat16)

    tri = np.triu(np.ones((128, 128), np.float32)).astype(bfloat16)
    # x quarters as SBUF images: xq[q][p, k*512+j] = x[b].T[k*128+p, q*512+j]
    xqs = []
    for b in range(B):
        xT = x[b].T.reshape(NK, 128, 4, 512)
        xqs.append([np.ascontiguousarray(
            xT[:, :, q, :].transpose(1, 0, 2).reshape(128, NK * 512)
        ).astype(bfloat16) for q in range(4)])
    in_maps = []
    for c in range(8):
        b, ho = c // 4, (c % 4) * NHL
        cols = slice(ho * HD, (ho + NHL) * HD)
        wv_aug = np.zeros((HID, VC), np.float32)
        wvl = np.zeros((1, VT), np.float32)
        for h in range(NHL):
            wv_aug[:, h * HS:h * HS + HD] = Wv[:, (ho + h) * HD:(ho + h + 1) * HD]
            wvl[0, h * HS:h * HS + HD] = bv[(ho + h) * HD:(ho + h + 1) * HD]
            wvl[0, h * HS + HD] = 1.0
        bqk_c = np.stack([bqk[:HID][cols][:128], bqk[:HID][cols][128:],
                          bqk[HID:][cols][:128], bqk[HID:][cols][128:]],
                         axis=1)
        m = {
            "wq": pack(Wqk[:, cols]),
            "wk": pack(Wqk[:, HID:][:, cols]),
            "wv": pack(wv_aug),
            "wvl": np.ascontiguousarray(
                np.broadcast_to(wvl, (128, VT))).astype(bfloat16),
            "bqk": np.ascontiguousarray(bqk_c.astype(np.float32)),
            "tri": tri,
        }
        for q in range(4):
            m[f"xq{q}"] = xqs[b][q]
        in_maps.append(m)
    return in_maps


def kernel(hidden_states, Wqk, bqk, Wv, bv):
    import time

    from concourse.bass_utils import run_bass_kernel_spmd

    in_maps = make_in_maps(hidden_states, Wqk, bqk, Wv, bv)
    res = None
    for attempt in range(3):
        try:
            res = run_bass_kernel_spmd(_get_nc(), in_maps, list(range(8)))
            break
        except Exception:
            # transient NRT_EXEC_UNIT_UNRECOVERABLE errors have been observed
            # on this fabric; back off and retry
            if attempt == 2:
                raise
            time.sleep(2.0)
    outp = np.empty((B, S, NH * HD), np.float32)
    for c in range(8):
        b, ho = c // 4, (c % 4) * NHL
        # out[r, ((a*2+ht)*2+hh)*512 + q]: head dim r (or denom at r=64)
        # of head 2*ht+hh, query a*512+q
        arr = res.results[c]["out"].astype(np.float32).reshape(
            HD + 1, NA, 2, 2, 512)
        blk = arr[:HD] / arr[HD]                      # [r, a, ht, hh, q]
        outp[b, :, ho * HD:(ho + NHL) * HD] = (
            blk.transpose(1, 4, 2, 3, 0).reshape(S, WC))
    return outp


# revision 29
# speedup vs baseline: 1.0230x; 1.0230x over previous
"""Causal self-attention (B=2, S=2048, HID=1024, 16 heads x 64) on 8 trn2
NeuronCores.

Sharding: data-parallel over batch (cores 0-3 -> batch 0, cores 4-7 ->
batch 1), tensor-parallel over heads (4 heads per core via Wqk/Wv column
slices). Each core computes its 4 heads end-to-end; the [S, S] score
matrix stays core-local.

Per-core layout choices:
  - All matmul operands are bf16 (inputs are cast host-side); PSUM
    accumulation stays fp32.
  - q, k are produced TRANSPOSED ([head_cols, S]) so score matmuls need
    no on-device transposes; scores are computed transposed ([sk, sq])
    so the P @ v matmul consumes exp(scores) directly from SBUF.
  - v carries an appended ones-column per head; the attention output
    matmul then yields softmax row-sums in an extra partition row for
    free (no max-subtraction is needed: scores are O(5) so exp is safe
    in fp32, and masked entries are zeroed multiplicatively post-exp
    with a GPSIMD multiply against a [128,128] triangle mask input).
  - EVERY stationary operand is a 4B-aligned, 128-column bf16 slice so
    the compiler emits FWL (fast weight load) LDWEIGHTS: FWL loads go
    to the background weight buffer and hide behind the running matmul
    stream; non-FWL loads serialize (~130ns each, measured). For P @ v
    this means v head slots are 66 cols (64 + ones + pad) and the lhsT
    is the OVERLAPPING 128-col window starting at each head slot - the
    extra columns produce garbage PSUM rows 66-127 that are never read.
  - v bias is a free DVE add (bias pre-broadcast host-side) fused into
    the PSUM->SBUF copy, replacing a K=1 matmul on the PE.
  - Heads are processed in pairs: the two K=64 score matmuls sit in PE
    row-groups 0-63 / 64-127 and run concurrently in the array.
  - The P @ v matmuls run 2 chunks BEHIND the score/exp stream, so the
    in-order PE queue never stalls on the ACT engine's exp latency.
  - Inputs arrive host-prepacked in the exact SBUF image as large
    contiguous DMAs split across the two HWDGE issue queues (Sync +
    Scalar), ordered so the first projection unit's inputs (wq + both
    xq0 halves) land first on separate queues.
  - The output is stored in the SBUF-native layout ([128, a, c, 256],
    one fully contiguous 512KB DMA per stripe) and re-laid-out on the
    host; the previous row-major store scattered 512B segments and
    cost ~13us of post-compute DMA tail.
  - Stripes run 0,1,3,2 so the trailing region still has projection
    work as PE filler; pair-tail P @ v + PSUM drain + head finalization
    (transpose + normalize) are deferred into the following units via a
    priority queue.
"""
import sys

for _p in ("/opt/trn_rl_repo",):
    if _p not in sys.path:
        sys.path.insert(0, _p)

import numpy as np

B, S, HID = 2, 2048, 1024
NH, HD = 16, 64
NHL = 4            # heads per core
WC = NHL * HD      # 256 local q/k weight cols
HS = HD + 2        # 66-col head slot in v: 64 + ones col + pad (4B align)
VC = NHL * HS      # 264 local v cols
VPAD = 3 * HS + 128  # 326: last head's 128-col lhsT window end
VT = 328           # padded v tile width (even)
NT = S // 128      # 16 key chunks
NA = S // 512      # 4 query stripes
NK = HID // 128    # 8 contraction chunks
LAG = 3            # P @ v trails the score/exp stream by this many chunks
NWARM = 6          # scratch warmup matmuls bridging the input DMA front

_NC = None


def _build():
    from concourse import bacc, mybir
    from concourse.tile import TileContext

    FP = mybir.dt.float32
    BF = mybir.dt.bfloat16
    Exp = mybir.ActivationFunctionType.Exp

    nc = bacc.Bacc("TRN2", target_bir_lowering=False, debug=False, num_devices=8)

    # all inputs are host-prepacked into the exact SBUF image, so every
    # input DMA is a fully-contiguous identity copy
    xq_d = [nc.dram_tensor(f"xq{q}", [128, NK * 512], BF, kind="ExternalInput")
            for q in range(4)]
    wq = nc.dram_tensor("wq", [128, NK * WC], BF, kind="ExternalInput")
    wk = nc.dram_tensor("wk", [128, NK * WC], BF, kind="ExternalInput")
    wv = nc.dram_tensor("wv", [128, NK * VC], BF, kind="ExternalInput")
    wvl_d = nc.dram_tensor("wvl", [128, VT], BF, kind="ExternalInput")
    bqk = nc.dram_tensor("bqk", [128, 4], FP, kind="ExternalInput")
    tri_d = nc.dram_tensor("tri", [128, 128], BF, kind="ExternalInput")
    # output leaves as the raw UNNORMALIZED [65, 512] P@v blocks (64 head
    # dims + softmax-denominator row) in bf16: the host does the divide +
    # transpose (trivial numpy). This removes all 64 PE transposes, the
    # DVE recip/normalize chain, and the identity input; bf16 adds ~0.2%
    # rounding against a 2e-2 rel-err budget.
    out = nc.dram_tensor("out", [HD + 1, NA * 2 * 1024], BF,
                         kind="ExternalOutput")

    with TileContext(nc) as tc:
        with (
            tc.tile_pool(name="inp", bufs=1) as inp,
            tc.tile_pool(name="ptp", bufs=8) as ptp,
            tc.tile_pool(name="osb", bufs=4) as osb,
            tc.tile_pool(name="G", bufs=3, space="PSUM") as gp,
            tc.tile_pool(name="oT", bufs=2, space="PSUM") as otp,
        ):
            # PE warmup on a scratch tile (no DMA dependency, so it starts
            # right after the engine preambles): keeps the PE busy through
            # the HAM SHORT window while the input DMA streams, so the
            # projection stream starts at 2.4 GHz instead of 1.2. memset
            # on gpsimd - its preamble finishes ~1us before the DVE's.
            scratch = inp.tile([128, 512], BF, name="scratch")
            nc.gpsimd.memset(scratch[:, :], 0.0)
            # preload the GPSIMD ucode IRAM (~6us, hidden in the preamble)
            # so the first in-stream gpsimd mask-multiply doesn't pay it
            gsc = inp.tile([128, 1], BF, name="gsc")
            nc.gpsimd.memset(gsc[:, :], 0.0)
            warm = gp.tile([128, 1024], mybir.dt.float32, tag="G", name="warm")
            for _ in range(NWARM):
                nc.tensor.matmul(warm[:, :512], lhsT=scratch[:, :128],
                                 rhs=scratch[:, :], start=True, stop=True)

            # ---- persistent inputs in SBUF, contiguous identity DMAs
            # split across the two HWDGE issue queues (Sync + Scalar).
            # The first projection unit needs wq + BOTH xq0 halves; wk
            # follows ~2us later, wv ~2us after that. Each queue moves
            # ~180 GB/s, so pair the critical tensors across queues.
            # critical-path order: wq/xq0_0 first on their queues (first
            # q half-unit), then wk / xq0_1 (k half + q second half), then
            # wv (v units, consumed ~20us in), tri (first boundary mask,
            # ~21us), xq1 (fillers from uidx 4), ident (first finish_head,
            # ~26us), and the remaining x quarters.
            xq = [[None, None] for _ in range(4)]
            # wq/wk and the xq0 halves are DMA'd in 256KB quarter-slices,
            # interleaved across the two queues in first-consumer order:
            # the queues move ~110-130GB/s each, the Tile dep tracker is
            # region-exact, and phase 1 below consumes in exactly this
            # order, so the PE starts ~10.5us in and is never more than
            # one 256KB slice away from its next unlocked work.
            wq_sb = inp.tile([128, NK * WC], BF, name="wq")
            wk_sb = inp.tile([128, NK * WC], BF, name="wk")
            xq[0][0] = inp.tile([128, 4 * 512], BF, name="xq0_0")
            xq[0][1] = inp.tile([128, 4 * 512], BF, name="xq0_1")
            nc.sync.dma_start(wq_sb[:, :4 * WC], wq[:, :4 * WC])
            nc.scalar.dma_start(xq[0][0][:, :1024], xq_d[0][:, 0:1024])
            nc.sync.dma_start(wq_sb[:, 4 * WC:], wq[:, 4 * WC:])
            nc.scalar.dma_start(xq[0][0][:, 1024:], xq_d[0][:, 1024:2048])
            nc.sync.dma_start(wk_sb[:, :4 * WC], wk[:, :4 * WC])
            nc.scalar.dma_start(xq[0][1][:, :1024], xq_d[0][:, 2048:3072])
            nc.sync.dma_start(wk_sb[:, 4 * WC:], wk[:, 4 * WC:])
            nc.scalar.dma_start(xq[0][1][:, 1024:], xq_d[0][:, 3072:4096])
            bqk_sb = inp.tile([128, 4], FP, name="bqk")
            nc.sync.dma_start(bqk_sb[:, :], bqk[:, :])
            wv_sb = inp.tile([128, NK * VC], BF, name="wv")
            nc.scalar.dma_start(wv_sb[:, :], wv[:, :])
            tri = inp.tile([128, 128], BF, name="tri")
            nc.scalar.dma_start(tri[:, :], tri_d[:, :])
            wvl_sb = inp.tile([128, VT], BF, name="wvl")
            nc.scalar.dma_start(wvl_sb[:, :], wvl_d[:, :])
            xq[1][0] = inp.tile([128, NK * 512], BF, name="xq1")
            nc.sync.dma_start(xq[1][0][:, :], xq_d[1][:, :])
            for qtr, eng in ((2, nc.scalar), (3, nc.sync)):
                t = inp.tile([128, NK * 512], BF, name=f"xq{qtr}")
                eng.dma_start(t[:, :], xq_d[qtr][:, :])
                xq[qtr][0] = t

            def xk(k, qtr):
                if qtr == 0:
                    t = xq[0][k // 4]
                    return t[:, (k % 4) * 512:(k % 4 + 1) * 512]
                return xq[qtr][0][:, k * 512:(k + 1) * 512]

            # split by S-quarter so interleaved later-quarter projection
            # writes can't false-depend against earlier attention reads
            qT_sb = [[inp.tile([128, 512], BF, name=f"qT{t}_{n}")
                      for n in range(4)] for t in range(2)]
            kT_sb = [[inp.tile([128, 512], BF, name=f"kT{t}_{n}")
                      for n in range(4)] for t in range(2)]
            v_sb = [inp.tile([128, VT], BF, name=f"v{c}") for c in range(NT)]

            # ---- projection emitters ----
            def proj_qk_unit(wt, bcol, dst, t, qtr):
                g = gp.tile([128, 1024], mybir.dt.float32, tag="G", name="g")
                for k in range(NK):
                    nc.tensor.matmul(
                        g[:, :512],
                        lhsT=wt[:, k * WC + t * 128:k * WC + (t + 1) * 128],
                        rhs=xk(k, qtr),
                        start=(k == 0), stop=(k == NK - 1),
                    )
                nc.vector.tensor_scalar_add(
                    dst[t][qtr][:, :], g[:, :512], bqk_sb[:, bcol + t:bcol + t + 1]
                )

            def proj_v_unit(c):
                qtr, cc = divmod(c, 4)
                g = gp.tile([128, 1024], mybir.dt.float32, tag="G", name="g")
                for k in range(NK):
                    nc.tensor.matmul(
                        g[:, :VC],
                        lhsT=xk(k, qtr)[:, cc * 128:(cc + 1) * 128],
                        rhs=wv_sb[:, k * VC:(k + 1) * VC],
                        start=(k == 0), stop=(k == NK - 1),
                    )
                # bias (+ ones col) via a free DVE add fused into the
                # PSUM->SBUF copy; cols VC..VT copy stale psum garbage to
                # keep the overlapping 128-col P@v weight windows in
                # written territory (their rows 66-127 are never read)
                nc.vector.tensor_add(v_sb[c][:, :], g[:, :VT], wvl_sb[:, :])

            # ---- attention emitters ----
            # score/exp for ONE key chunk b of a head PAIR:
            # g = [h0-slice | h1-slice], one exp covers both heads
            def score_exp_unit(a, ht, b):
                g = gp.tile([128, 1024], mybir.dt.float32, tag="G", name="g")
                kn, ko = divmod(b * 128, 512)
                # diagonal chunks: columns < off are fully masked -- skip
                # them in the score matmul, the exp, and the P @ v matmul
                off = max(0, (b - 4 * a) * 128)
                for hh in range(2):
                    hb = hh * 64
                    nc.tensor.matmul(
                        g[:, hh * 512 + off:(hh + 1) * 512],
                        lhsT=kT_sb[ht][kn][hb:hb + 64, ko:ko + 128],
                        rhs=qT_sb[ht][a][hb:hb + 64, off:],
                        start=True, stop=True,
                    )
                pt = ptp.tile([128, 1024], BF, tag="pt", name="pt")
                if off:
                    gv = g[:, :].rearrange("p (h w) -> p h w", h=2)[:, :, off:]
                    pv = pt[:, :].rearrange("p (h w) -> p h w", h=2)[:, :, off:]
                    nc.scalar.activation(pv, gv, Exp, scale=HD ** -0.5)
                else:
                    nc.scalar.activation(pt[:, :], g[:, :], Exp, scale=HD ** -0.5)
                if b >= 4 * a:
                    # triangular boundary block: multiplicative mask. On
                    # GPSIMD (otherwise idle) to keep the DVE queue short --
                    # DVE reads are what free PSUM slots for the PE.
                    for hh in range(2):
                        c0 = hh * 512 + off
                        nc.gpsimd.tensor_mul(
                            pt[:, c0:c0 + 128], pt[:, c0:c0 + 128], tri[:, :]
                        )
                return pt

            def av_unit(a, ht, b, nchunks, oTs, pt):
                off = max(0, (b - 4 * a) * 128)
                for hh in range(2):
                    h = 2 * ht + hh
                    # lhsT is the 128-col window starting at head h's v
                    # slot: cols 0-63 v, 64 ones, 65 pad, 66-127 the next
                    # heads' data -> out rows 66-127 garbage, never read.
                    # 128 cols + 4B-aligned base => FWL, load is hidden.
                    nc.tensor.matmul(
                        oTs[hh][:, off:],
                        lhsT=v_sb[b][:, h * HS:h * HS + 128],
                        rhs=pt[:, hh * 512 + off:(hh + 1) * 512],
                        start=(b == 0), stop=(b == nchunks - 1),
                    )

            # ---- phase 1: the minimum needed by stripe a=0 head pair 0.
            # q/k t=0 units are emitted in k-HALVES: the first 4 matmuls
            # need only wq + the first 512KB half of xq0 (per-queue DMA
            # moves ~130GB/s, so halving the critical bytes starts the
            # real stream ~4us earlier); the second halves then wait on
            # xq0's other half / wk, which land while the first run.
            def proj_qk_half(wt, bcol, dst, t, ks, g):
                for k in ks:
                    nc.tensor.matmul(
                        g[:, :512],
                        lhsT=wt[:, k * WC + t * 128:k * WC + (t + 1) * 128],
                        rhs=xk(k, 0),
                        start=(k == 0), stop=(k == NK - 1),
                    )
                if ks[-1] == NK - 1:
                    nc.vector.tensor_scalar_add(
                        dst[t][0][:, :], g[:, :512], bqk_sb[:, bcol + t:bcol + t + 1]
                    )

            def scratch_mms(n):
                # elastic PE filler between DMA-paced quarter-units: costs
                # nothing when the next slice is late, delays at most its
                # own length when the slice is early
                for _ in range(n):
                    nc.tensor.matmul(warm[:, :512], lhsT=scratch[:, :128],
                                     rhs=scratch[:, :], start=True, stop=True)

            gq = gp.tile([128, 1024], mybir.dt.float32, tag="G", name="g")
            gk = gp.tile([128, 1024], mybir.dt.float32, tag="G", name="g")
            proj_qk_half(wq_sb, 0, qT_sb, 0, [0, 1], gq)
            scratch_mms(4)
            proj_qk_half(wq_sb, 0, qT_sb, 0, [2, 3], gq)
            scratch_mms(4)
            proj_qk_half(wk_sb, 2, kT_sb, 0, [0, 1], gk)
            scratch_mms(3)
            proj_qk_half(wk_sb, 2, kT_sb, 0, [2, 3], gk)
            scratch_mms(2)
            proj_qk_half(wq_sb, 0, qT_sb, 0, [4, 5], gq)
            proj_qk_half(wk_sb, 2, kT_sb, 0, [4, 5], gk)
            proj_qk_half(wq_sb, 0, qT_sb, 0, [6, 7], gq)
            proj_qk_half(wk_sb, 2, kT_sb, 0, [6, 7], gk)
            # v units 0/1 are NOT emitted here: the first scores go first,
            # so the exp stream starts ~2us earlier and recycles the G
            # psum ring for the v/proj fillers (ring depth is 3; a filler
            # allocating before the first exp completes stalls the PE)

            # remaining projection units are doled out between attention
            # units, scheduled (just) before their first consumer, keeping
            # the PE busy while ACT works through the exp stream
            def q_(t, qtr):
                return lambda: proj_qk_unit(wq_sb, 0, qT_sb, t, qtr)

            def k_(t, qtr):
                return lambda: proj_qk_unit(wk_sb, 2, kT_sb, t, qtr)

            def v_(c):
                return lambda: proj_v_unit(c)

            # placement: just-before-first-consumer deadlines, spread so
            # every region keeps the PE slightly ahead of the exp stream.
            # Stripes run 0,1,3,2: the trailing stripe-2 region (24 units)
            # then still has its own q/k projections left as PE filler,
            # where stripe 3 last would leave the PE starved (and the HAM
            # clock gate re-throttling) for its final 16 units.
            # NOTE: stripe 3 consumes ALL kT quarters and v chunks, so only
            # the stripe-2 q projections can be held back for the tail
            filler = {
                1: [v_(0)], 2: [v_(1), q_(1, 0)], 3: [v_(2), k_(1, 0)],
                4: [v_(3)], 5: [q_(0, 1)], 6: [k_(0, 1)], 7: [v_(4)],
                8: [v_(5)], 10: [v_(6)], 12: [v_(7)], 13: [q_(1, 1)],
                15: [k_(1, 1)], 17: [q_(0, 3)], 19: [k_(0, 3)],
                21: [v_(8)], 23: [k_(0, 2)],
                25: [v_(9)], 26: [v_(10)], 28: [v_(11)], 30: [v_(12)],
                32: [v_(13)], 34: [v_(14)], 36: [v_(15)], 38: [q_(1, 3)],
                39: [k_(1, 3)], 44: [k_(1, 2)], 50: [q_(0, 2)],
                58: [q_(1, 2)],
            }

            deferred = []          # drain/store closures fed into the stream

            # ---- phases 2+3: attention, software-pipelined ----
            uidx = 0
            for a in (0, 1, 3, 2):
                nchunks = 4 * a + 4
                for ht in range(2):
                    oTs = [otp.tile([128, 512], mybir.dt.float32,
                                    tag="oT", name="oT") for _ in range(2)]
                    pend = []
                    for b in range(nchunks):
                        pend.append((b, score_exp_unit(a, ht, b)))
                        if len(pend) > LAG:
                            bb, pt = pend.pop(0)
                            av_unit(a, ht, bb, nchunks, oTs, pt)
                        for f in filler.get(uidx, ()):
                            f()
                        for _ in range(2 if len(deferred) > 6 else 1):
                            if deferred:
                                deferred.pop(0)()
                        uidx += 1
                    # the tail P @ v matmuls and the oT psum drain flow into
                    # the NEXT pair's units (via the priority end of the
                    # deferred queue): by then the exp stream has caught up,
                    # so the in-order PE queue never waits at pair boundaries
                    bb0, pt0 = pend.pop(0)
                    av_unit(a, ht, bb0, nchunks, oTs, pt0)

                    def tail_av(a_=a, ht_=ht, p_=tuple(pend), n_=nchunks,
                                o_=oTs):
                        for bb, pt in p_:
                            av_unit(a_, ht_, bb, n_, o_, pt)

                    def drain(a_=a, ht_=ht, o_=oTs):
                        # pack both heads' raw [65, 512] blocks side by side
                        # and ship them as ONE contiguous 2KB-per-partition
                        # DMA; the host divides by the denominator row and
                        # transposes
                        ot = osb.tile([HD + 1, 1024], BF, tag="oTsb",
                                      name="oTsb")
                        for hh in range(2):
                            nc.vector.tensor_copy(
                                ot[:, hh * 512:(hh + 1) * 512],
                                o_[hh][:HD + 1, :])

                        def store(a_=a_, ht_=ht_, t_=ot):
                            nc.sync.dma_start(
                                out[:, (a_ * 2 + ht_) * 1024:
                                    (a_ * 2 + ht_ + 1) * 1024],
                                t_[:, :])
                        deferred.append(store)

                    deferred.insert(0, drain)
                    deferred.insert(0, tail_av)
            while deferred:
                deferred.pop(0)()

    nc.compile()
    return nc


def _get_nc():
    global _NC
    if _NC is None:
        _NC = _build()
    return _NC


def make_in_maps(hidden_states, Wqk, bqk, Wv, bv):
    from ml_dtypes import bfloat16

    x = np.asarray(hidden_states, dtype=np.float32)
    Wqk = np.asarray(Wqk, dtype=np.float32)
    bqk = np.asarray(bqk, dtype=np.float32)
    Wv = np.asarray(Wv, dtype=np.float32)
    bv = np.asarray(bv, dtype=np.float32)

    def pack(w):
        # [1024, C] -> SBUF image [128, 8*C] (k-chunk-major columns)
        c = w.shape[1]
        return np.ascontiguousarray(
            w.reshape(NK, 128, c).transpose(1, 0, 2).reshape(128, NK * c)
        ).astype(bfloat16)

    tri = np.triu(np.ones((128, 128), np.float32)).astype(bfloat16)
    # x quarters as SBUF images: xq[q][p, k*512+j] = x[b].T[k*128+p, q*512+j]
    xqs = []
    for b in range(B):
        xT = x[b].T.reshape(NK, 128, 4, 512)
        xqs.append([np.ascontiguousarray(
            xT[:, :, q, :].transpose(1, 0, 2).reshape(128, NK * 512)
        ).astype(bfloat16) for q in range(4)])
    in_maps = []
    for c in range(8):
        b, ho = c // 4, (c % 4) * NHL
        cols = slice(ho * HD, (ho + NHL) * HD)
        wv_aug = np.zeros((HID, VC), np.float32)
        wvl = np.zeros((1, VT), np.float32)
        for h in range(NHL):
            wv_aug[:, h * HS:h * HS + HD] = Wv[:, (ho + h) * HD:(ho + h + 1) * HD]
            wvl[0, h * HS:h * HS + HD] = bv[(ho + h) * HD:(ho + h + 1) * HD]
            wvl[0, h * HS + HD] = 1.0
        bqk_c = np.stack([bqk[:HID][cols][:128], bqk[:HID][cols][128:],
                          bqk[HID:][cols][:128], bqk[HID:][cols][128:]],
                         axis=1)
        m = {
            "wq": pack(Wqk[:, cols]),
            "wk": pack(Wqk[:, HID:][:, cols]),
            "wv": pack(wv_aug),
            "wvl": np.ascontiguousarray(
                np.broadcast_to(wvl, (128, VT))).astype(bfloat16),
            "bqk": np.ascontiguousarray(bqk_c.astype(np.float32)),
            "tri": tri,
        }
        for q in range(4):
            m[f"xq{q}"] = xqs[b][q]
        in_maps.append(m)
    return in_maps


def kernel(hidden_states, Wqk, bqk, Wv, bv):
    import time

    from concourse.bass_utils import run_bass_kernel_spmd

    in_maps = make_in_maps(hidden_states, Wqk, bqk, Wv, bv)
    res = None
    for attempt in range(3):
        try:
            res = run_bass_kernel_spmd(_get_nc(), in_maps, list(range(8)))
            break
        except Exception:
            # transient NRT_EXEC_UNIT_UNRECOVERABLE errors have been observed
            # on this fabric; back off and retry
            if attempt == 2:
                raise
            time.sleep(2.0)
    outp = np.empty((B, S, NH * HD), np.float32)
    for c in range(8):
        b, ho = c // 4, (c % 4) * NHL
        # out[r, ((a*2+ht)*2+hh)*512 + q]: head dim r (or denom at r=64)
        # of head 2*ht+hh, query a*512+q
        arr = res.results[c]["out"].astype(np.float32).reshape(
            HD + 1, NA, 2, 2, 512)
        blk = arr[:HD] / arr[HD]                      # [r, a, ht, hh, q]
        outp[b, :, ho * HD:(ho + NHL) * HD] = (
            blk.transpose(1, 4, 2, 3, 0).reshape(S, WC))
    return outp


# revision 30
# speedup vs baseline: 1.0303x; 1.0071x over previous
"""Causal self-attention (B=2, S=2048, HID=1024, 16 heads x 64) on 8 trn2
NeuronCores.

Sharding: data-parallel over batch (cores 0-3 -> batch 0, cores 4-7 ->
batch 1), tensor-parallel over heads (4 heads per core via Wqk/Wv column
slices). Each core computes its 4 heads end-to-end; the [S, S] score
matrix stays core-local.

Per-core layout choices:
  - All matmul operands are bf16 (inputs are cast host-side); PSUM
    accumulation stays fp32.
  - q, k are produced TRANSPOSED ([head_cols, S]) so score matmuls need
    no on-device transposes; scores are computed transposed ([sk, sq])
    so the P @ v matmul consumes exp(scores) directly from SBUF.
  - v carries an appended ones-column per head; the attention output
    matmul then yields softmax row-sums in an extra partition row for
    free (no max-subtraction is needed: scores are O(5) so exp is safe
    in fp32, and masked entries are zeroed multiplicatively post-exp
    with a GPSIMD multiply against a [128,128] triangle mask input).
  - EVERY stationary operand is a 4B-aligned, 128-column bf16 slice so
    the compiler emits FWL (fast weight load) LDWEIGHTS: FWL loads go
    to the background weight buffer and hide behind the running matmul
    stream; non-FWL loads serialize (~130ns each, measured). For P @ v
    this means v head slots are 66 cols (64 + ones + pad) and the lhsT
    is the OVERLAPPING 128-col window starting at each head slot - the
    extra columns produce garbage PSUM rows 66-127 that are never read.
  - v bias is a free DVE add (bias pre-broadcast host-side) fused into
    the PSUM->SBUF copy, replacing a K=1 matmul on the PE.
  - Heads are processed in pairs: the two K=64 score matmuls sit in PE
    row-groups 0-63 / 64-127 and run concurrently in the array.
  - The P @ v matmuls run 2 chunks BEHIND the score/exp stream, so the
    in-order PE queue never stalls on the ACT engine's exp latency.
  - Inputs arrive host-prepacked in the exact SBUF image as large
    contiguous DMAs split across the two HWDGE issue queues (Sync +
    Scalar), ordered so the first projection unit's inputs (wq + both
    xq0 halves) land first on separate queues.
  - The output is stored in the SBUF-native layout ([128, a, c, 256],
    one fully contiguous 512KB DMA per stripe) and re-laid-out on the
    host; the previous row-major store scattered 512B segments and
    cost ~13us of post-compute DMA tail.
  - Stripes run 0,1,3,2 so the trailing region still has projection
    work as PE filler; pair-tail P @ v + PSUM drain + head finalization
    (transpose + normalize) are deferred into the following units via a
    priority queue.
"""
import sys

for _p in ("/opt/trn_rl_repo",):
    if _p not in sys.path:
        sys.path.insert(0, _p)

import numpy as np

B, S, HID = 2, 2048, 1024
NH, HD = 16, 64
NHL = 4            # heads per core
WC = NHL * HD      # 256 local q/k weight cols
HS = HD + 2        # 66-col head slot in v: 64 + ones col + pad (4B align)
VC = NHL * HS      # 264 local v cols
VPAD = 3 * HS + 128  # 326: last head's 128-col lhsT window end
VT = 328           # padded v tile width (even)
NT = S // 128      # 16 key chunks
NA = S // 512      # 4 query stripes
NK = HID // 128    # 8 contraction chunks
LAG = 3            # P @ v trails the score/exp stream by this many chunks
NWARM = 12         # scratch warmup matmuls bridging the input DMA front

_NC = None


def _build():
    from concourse import bacc, mybir
    from concourse.tile import TileContext

    FP = mybir.dt.float32
    BF = mybir.dt.bfloat16
    Exp = mybir.ActivationFunctionType.Exp

    nc = bacc.Bacc("TRN2", target_bir_lowering=False, debug=False, num_devices=8)

    # all inputs are host-prepacked into the exact SBUF image, so every
    # input DMA is a fully-contiguous identity copy
    xq_d = [nc.dram_tensor(f"xq{q}", [128, NK * 512], BF, kind="ExternalInput")
            for q in range(4)]
    wq = nc.dram_tensor("wq", [128, NK * WC], BF, kind="ExternalInput")
    wk = nc.dram_tensor("wk", [128, NK * WC], BF, kind="ExternalInput")
    wv = nc.dram_tensor("wv", [128, NK * VC], BF, kind="ExternalInput")
    wvl_d = nc.dram_tensor("wvl", [128, VT], BF, kind="ExternalInput")
    bqk = nc.dram_tensor("bqk", [128, 4], FP, kind="ExternalInput")
    tri_d = nc.dram_tensor("tri", [128, 128], BF, kind="ExternalInput")
    # output leaves as the raw UNNORMALIZED [65, 512] P@v blocks (64 head
    # dims + softmax-denominator row) in bf16: the host does the divide +
    # transpose (trivial numpy). This removes all 64 PE transposes, the
    # DVE recip/normalize chain, and the identity input; bf16 adds ~0.2%
    # rounding against a 2e-2 rel-err budget.
    out = nc.dram_tensor("out", [HD + 1, NA * 2 * 1024], BF,
                         kind="ExternalOutput")

    with TileContext(nc) as tc:
        with (
            tc.tile_pool(name="inp", bufs=1) as inp,
            tc.tile_pool(name="ptp", bufs=8) as ptp,
            tc.tile_pool(name="osb", bufs=4) as osb,
            tc.tile_pool(name="G", bufs=3, space="PSUM") as gp,
            tc.tile_pool(name="oT", bufs=2, space="PSUM") as otp,
        ):
            # PE warmup on a scratch tile (no DMA dependency, so it starts
            # right after the engine preambles): keeps the PE busy through
            # the HAM SHORT window while the input DMA streams, so the
            # projection stream starts at 2.4 GHz instead of 1.2. memset
            # on gpsimd - its preamble finishes ~1us before the DVE's.
            scratch = inp.tile([128, 512], BF, name="scratch")
            nc.gpsimd.memset(scratch[:, :], 0.0)
            # preload the GPSIMD ucode IRAM (~6us, hidden in the preamble)
            # so the first in-stream gpsimd mask-multiply doesn't pay it
            gsc = inp.tile([128, 1], BF, name="gsc")
            nc.gpsimd.memset(gsc[:, :], 0.0)
            warm = gp.tile([128, 1024], mybir.dt.float32, tag="G", name="warm")
            for _ in range(NWARM):
                nc.tensor.matmul(warm[:, :512], lhsT=scratch[:, :128],
                                 rhs=scratch[:, :], start=True, stop=True)

            # ---- persistent inputs in SBUF, contiguous identity DMAs
            # split across the two HWDGE issue queues (Sync + Scalar).
            # The first projection unit needs wq + BOTH xq0 halves; wk
            # follows ~2us later, wv ~2us after that. Each queue moves
            # ~180 GB/s, so pair the critical tensors across queues.
            # critical-path order: wq/xq0_0 first on their queues (first
            # q half-unit), then wk / xq0_1 (k half + q second half), then
            # wv (v units, consumed ~20us in), tri (first boundary mask,
            # ~21us), xq1 (fillers from uidx 4), ident (first finish_head,
            # ~26us), and the remaining x quarters.
            xq = [[None, None] for _ in range(4)]
            # wq/wk and the xq0 halves are DMA'd in 256KB quarter-slices,
            # interleaved across the two queues in first-consumer order:
            # the queues move ~110-130GB/s each, the Tile dep tracker is
            # region-exact, and phase 1 below consumes in exactly this
            # order, so the PE starts ~10.5us in and is never more than
            # one 256KB slice away from its next unlocked work.
            wq_sb = inp.tile([128, NK * WC], BF, name="wq")
            wk_sb = inp.tile([128, NK * WC], BF, name="wk")
            xq[0][0] = inp.tile([128, 4 * 512], BF, name="xq0_0")
            xq[0][1] = inp.tile([128, 4 * 512], BF, name="xq0_1")
            nc.sync.dma_start(wq_sb[:, :4 * WC], wq[:, :4 * WC])
            nc.scalar.dma_start(xq[0][0][:, :1024], xq_d[0][:, 0:1024])
            nc.sync.dma_start(wq_sb[:, 4 * WC:], wq[:, 4 * WC:])
            nc.scalar.dma_start(xq[0][0][:, 1024:], xq_d[0][:, 1024:2048])
            nc.sync.dma_start(wk_sb[:, :4 * WC], wk[:, :4 * WC])
            nc.scalar.dma_start(xq[0][1][:, :1024], xq_d[0][:, 2048:3072])
            nc.sync.dma_start(wk_sb[:, 4 * WC:], wk[:, 4 * WC:])
            nc.scalar.dma_start(xq[0][1][:, 1024:], xq_d[0][:, 3072:4096])
            bqk_sb = inp.tile([128, 4], FP, name="bqk")
            nc.sync.dma_start(bqk_sb[:, :], bqk[:, :])
            wv_sb = inp.tile([128, NK * VC], BF, name="wv")
            nc.scalar.dma_start(wv_sb[:, :], wv[:, :])
            tri = inp.tile([128, 128], BF, name="tri")
            nc.scalar.dma_start(tri[:, :], tri_d[:, :])
            wvl_sb = inp.tile([128, VT], BF, name="wvl")
            nc.scalar.dma_start(wvl_sb[:, :], wvl_d[:, :])
            xq[1][0] = inp.tile([128, NK * 512], BF, name="xq1")
            nc.sync.dma_start(xq[1][0][:, :], xq_d[1][:, :])
            for qtr, eng in ((2, nc.scalar), (3, nc.sync)):
                t = inp.tile([128, NK * 512], BF, name=f"xq{qtr}")
                eng.dma_start(t[:, :], xq_d[qtr][:, :])
                xq[qtr][0] = t

            def xk(k, qtr):
                if qtr == 0:
                    t = xq[0][k // 4]
                    return t[:, (k % 4) * 512:(k % 4 + 1) * 512]
                return xq[qtr][0][:, k * 512:(k + 1) * 512]

            # split by S-quarter so interleaved later-quarter projection
            # writes can't false-depend against earlier attention reads
            qT_sb = [[inp.tile([128, 512], BF, name=f"qT{t}_{n}")
                      for n in range(4)] for t in range(2)]
            kT_sb = [[inp.tile([128, 512], BF, name=f"kT{t}_{n}")
                      for n in range(4)] for t in range(2)]
            v_sb = [inp.tile([128, VT], BF, name=f"v{c}") for c in range(NT)]

            # ---- projection emitters ----
            def proj_qk_unit(wt, bcol, dst, t, qtr):
                g = gp.tile([128, 1024], mybir.dt.float32, tag="G", name="g")
                for k in range(NK):
                    nc.tensor.matmul(
                        g[:, :512],
                        lhsT=wt[:, k * WC + t * 128:k * WC + (t + 1) * 128],
                        rhs=xk(k, qtr),
                        start=(k == 0), stop=(k == NK - 1),
                    )
                nc.vector.tensor_scalar_add(
                    dst[t][qtr][:, :], g[:, :512], bqk_sb[:, bcol + t:bcol + t + 1]
                )

            def proj_v_unit(c):
                qtr, cc = divmod(c, 4)
                g = gp.tile([128, 1024], mybir.dt.float32, tag="G", name="g")
                for k in range(NK):
                    nc.tensor.matmul(
                        g[:, :VC],
                        lhsT=xk(k, qtr)[:, cc * 128:(cc + 1) * 128],
                        rhs=wv_sb[:, k * VC:(k + 1) * VC],
                        start=(k == 0), stop=(k == NK - 1),
                    )
                # bias (+ ones col) via a free DVE add fused into the
                # PSUM->SBUF copy; cols VC..VT copy stale psum garbage to
                # keep the overlapping 128-col P@v weight windows in
                # written territory (their rows 66-127 are never read)
                nc.vector.tensor_add(v_sb[c][:, :], g[:, :VT], wvl_sb[:, :])

            # ---- attention emitters ----
            # score/exp for ONE key chunk b of a head PAIR:
            # g = [h0-slice | h1-slice], one exp covers both heads
            def score_exp_unit(a, ht, b):
                g = gp.tile([128, 1024], mybir.dt.float32, tag="G", name="g")
                kn, ko = divmod(b * 128, 512)
                # diagonal chunks: columns < off are fully masked -- skip
                # them in the score matmul, the exp, and the P @ v matmul
                off = max(0, (b - 4 * a) * 128)
                for hh in range(2):
                    hb = hh * 64
                    nc.tensor.matmul(
                        g[:, hh * 512 + off:(hh + 1) * 512],
                        lhsT=kT_sb[ht][kn][hb:hb + 64, ko:ko + 128],
                        rhs=qT_sb[ht][a][hb:hb + 64, off:],
                        start=True, stop=True,
                    )
                pt = ptp.tile([128, 1024], BF, tag="pt", name="pt")
                if off:
                    gv = g[:, :].rearrange("p (h w) -> p h w", h=2)[:, :, off:]
                    pv = pt[:, :].rearrange("p (h w) -> p h w", h=2)[:, :, off:]
                    nc.scalar.activation(pv, gv, Exp, scale=HD ** -0.5)
                else:
                    nc.scalar.activation(pt[:, :], g[:, :], Exp, scale=HD ** -0.5)
                if b >= 4 * a:
                    # triangular boundary block: multiplicative mask. On
                    # GPSIMD (otherwise idle) to keep the DVE queue short --
                    # DVE reads are what free PSUM slots for the PE.
                    for hh in range(2):
                        c0 = hh * 512 + off
                        nc.gpsimd.tensor_mul(
                            pt[:, c0:c0 + 128], pt[:, c0:c0 + 128], tri[:, :]
                        )
                return pt

            def av_unit(a, ht, b, nchunks, oTs, pt):
                off = max(0, (b - 4 * a) * 128)
                for hh in range(2):
                    h = 2 * ht + hh
                    # lhsT is the 128-col window starting at head h's v
                    # slot: cols 0-63 v, 64 ones, 65 pad, 66-127 the next
                    # heads' data -> out rows 66-127 garbage, never read.
                    # 128 cols + 4B-aligned base => FWL, load is hidden.
                    nc.tensor.matmul(
                        oTs[hh][:, off:],
                        lhsT=v_sb[b][:, h * HS:h * HS + 128],
                        rhs=pt[:, hh * 512 + off:(hh + 1) * 512],
                        start=(b == 0), stop=(b == nchunks - 1),
                    )

            # ---- phase 1: the minimum needed by stripe a=0 head pair 0.
            # q/k t=0 units are emitted in k-HALVES: the first 4 matmuls
            # need only wq + the first 512KB half of xq0 (per-queue DMA
            # moves ~130GB/s, so halving the critical bytes starts the
            # real stream ~4us earlier); the second halves then wait on
            # xq0's other half / wk, which land while the first run.
            def proj_qk_half(wt, bcol, dst, t, ks, g):
                for k in ks:
                    nc.tensor.matmul(
                        g[:, :512],
                        lhsT=wt[:, k * WC + t * 128:k * WC + (t + 1) * 128],
                        rhs=xk(k, 0),
                        start=(k == 0), stop=(k == NK - 1),
                    )
                if ks[-1] == NK - 1:
                    nc.vector.tensor_scalar_add(
                        dst[t][0][:, :], g[:, :512], bqk_sb[:, bcol + t:bcol + t + 1]
                    )

            def scratch_mms(n):
                # elastic PE filler between DMA-paced quarter-units: costs
                # nothing when the next slice is late, delays at most its
                # own length when the slice is early
                for _ in range(n):
                    nc.tensor.matmul(warm[:, :512], lhsT=scratch[:, :128],
                                     rhs=scratch[:, :], start=True, stop=True)

            gq = gp.tile([128, 1024], mybir.dt.float32, tag="G", name="g")
            gk = gp.tile([128, 1024], mybir.dt.float32, tag="G", name="g")
            proj_qk_half(wq_sb, 0, qT_sb, 0, [0, 1], gq)
            scratch_mms(4)
            proj_qk_half(wq_sb, 0, qT_sb, 0, [2, 3], gq)
            scratch_mms(4)
            proj_qk_half(wk_sb, 2, kT_sb, 0, [0, 1], gk)
            scratch_mms(3)
            proj_qk_half(wk_sb, 2, kT_sb, 0, [2, 3], gk)
            scratch_mms(2)
            proj_qk_half(wq_sb, 0, qT_sb, 0, [4, 5], gq)
            proj_qk_half(wk_sb, 2, kT_sb, 0, [4, 5], gk)
            proj_qk_half(wq_sb, 0, qT_sb, 0, [6, 7], gq)
            proj_qk_half(wk_sb, 2, kT_sb, 0, [6, 7], gk)
            # v units 0/1 are NOT emitted here: the first scores go first,
            # so the exp stream starts ~2us earlier and recycles the G
            # psum ring for the v/proj fillers (ring depth is 3; a filler
            # allocating before the first exp completes stalls the PE)

            # remaining projection units are doled out between attention
            # units, scheduled (just) before their first consumer, keeping
            # the PE busy while ACT works through the exp stream
            def q_(t, qtr):
                return lambda: proj_qk_unit(wq_sb, 0, qT_sb, t, qtr)

            def k_(t, qtr):
                return lambda: proj_qk_unit(wk_sb, 2, kT_sb, t, qtr)

            def v_(c):
                return lambda: proj_v_unit(c)

            # placement: just-before-first-consumer deadlines, spread so
            # every region keeps the PE slightly ahead of the exp stream.
            # Stripes run 0,1,3,2: the trailing stripe-2 region (24 units)
            # then still has its own q/k projections left as PE filler,
            # where stripe 3 last would leave the PE starved (and the HAM
            # clock gate re-throttling) for its final 16 units.
            # NOTE: stripe 3 consumes ALL kT quarters and v chunks, so only
            # the stripe-2 q projections can be held back for the tail
            filler = {
                1: [v_(0)], 2: [v_(1), q_(1, 0)], 3: [v_(2), k_(1, 0)],
                4: [v_(3)], 5: [q_(0, 1)], 6: [k_(0, 1)], 7: [v_(4)],
                8: [v_(5)], 10: [v_(6)], 12: [v_(7)], 13: [q_(1, 1)],
                15: [k_(1, 1)], 17: [q_(0, 3)], 19: [k_(0, 3)],
                21: [v_(8)], 23: [k_(0, 2)],
                25: [v_(9)], 26: [v_(10)], 28: [v_(11)], 30: [v_(12)],
                32: [v_(13)], 34: [v_(14)], 36: [v_(15)], 38: [q_(1, 3)],
                39: [k_(1, 3)], 44: [k_(1, 2)], 50: [q_(0, 2)],
                58: [q_(1, 2)],
            }

            deferred = []          # drain/store closures fed into the stream

            # ---- phases 2+3: attention, software-pipelined ----
            uidx = 0
            for a in (0, 1, 3, 2):
                nchunks = 4 * a + 4
                for ht in range(2):
                    oTs = [otp.tile([128, 512], mybir.dt.float32,
                                    tag="oT", name="oT") for _ in range(2)]
                    pend = []
                    for b in range(nchunks):
                        pend.append((b, score_exp_unit(a, ht, b)))
                        if len(pend) > LAG:
                            bb, pt = pend.pop(0)
                            av_unit(a, ht, bb, nchunks, oTs, pt)
                        for f in filler.get(uidx, ()):
                            f()
                        for _ in range(2 if len(deferred) > 6 else 1):
                            if deferred:
                                deferred.pop(0)()
                        uidx += 1
                    # the tail P @ v matmuls and the oT psum drain flow into
                    # the NEXT pair's units (via the priority end of the
                    # deferred queue): by then the exp stream has caught up,
                    # so the in-order PE queue never waits at pair boundaries
                    bb0, pt0 = pend.pop(0)
                    av_unit(a, ht, bb0, nchunks, oTs, pt0)

                    def tail_av(a_=a, ht_=ht, p_=tuple(pend), n_=nchunks,
                                o_=oTs):
                        for bb, pt in p_:
                            av_unit(a_, ht_, bb, n_, o_, pt)

                    def drain(a_=a, ht_=ht, o_=oTs):
                        # pack both heads' raw [65, 512] blocks side by side
                        # and ship them as ONE contiguous 2KB-per-partition
                        # DMA; the host divides by the denominator row and
                        # transposes
                        ot = osb.tile([HD + 1, 1024], BF, tag="oTsb",
                                      name="oTsb")
                        for hh in range(2):
                            nc.vector.tensor_copy(
                                ot[:, hh * 512:(hh + 1) * 512],
                                o_[hh][:HD + 1, :])

                        def store(a_=a_, ht_=ht_, t_=ot):
                            nc.sync.dma_start(
                                out[:, (a_ * 2 + ht_) * 1024:
                                    (a_ * 2 + ht_ + 1) * 1024],
                                t_[:, :])
                        deferred.append(store)

                    deferred.insert(0, drain)
                    deferred.insert(0, tail_av)
            while deferred:
                deferred.pop(0)()

    nc.compile()
    return nc


def _get_nc():
    global _NC
    if _NC is None:
        _NC = _build()
    return _NC


def make_in_maps(hidden_states, Wqk, bqk, Wv, bv):
    from ml_dtypes import bfloat16

    x = np.asarray(hidden_states, dtype=np.float32)
    Wqk = np.asarray(Wqk, dtype=np.float32)
    bqk = np.asarray(bqk, dtype=np.float32)
    Wv = np.asarray(Wv, dtype=np.float32)
    bv = np.asarray(bv, dtype=np.float32)

    def pack(w):
        # [1024, C] -> SBUF image [128, 8*C] (k-chunk-major columns)
        c = w.shape[1]
        return np.ascontiguousarray(
            w.reshape(NK, 128, c).transpose(1, 0, 2).reshape(128, NK * c)
        ).astype(bfloat16)

    tri = np.triu(np.ones((128, 128), np.float32)).astype(bfloat16)
    # x quarters as SBUF images: xq[q][p, k*512+j] = x[b].T[k*128+p, q*512+j]
    xqs = []
    for b in range(B):
        xT = x[b].T.reshape(NK, 128, 4, 512)
        xqs.append([np.ascontiguousarray(
            xT[:, :, q, :].transpose(1, 0, 2).reshape(128, NK * 512)
        ).astype(bfloat16) for q in range(4)])
    in_maps = []
    for c in range(8):
        b, ho = c // 4, (c % 4) * NHL
        cols = slice(ho * HD, (ho + NHL) * HD)
        wv_aug = np.zeros((HID, VC), np.float32)
        wvl = np.zeros((1, VT), np.float32)
        for h in range(NHL):
            wv_aug[:, h * HS:h * HS + HD] = Wv[:, (ho + h) * HD:(ho + h + 1) * HD]
            wvl[0, h * HS:h * HS + HD] = bv[(ho + h) * HD:(ho + h + 1) * HD]
            wvl[0, h * HS + HD] = 1.0
        bqk_c = np.stack([bqk[:HID][cols][:128], bqk[:HID][cols][128:],
                          bqk[HID:][cols][:128], bqk[HID:][cols][128:]],
                         axis=1)
        m = {
            "wq": pack(Wqk[:, cols]),
            "wk": pack(Wqk[:, HID:][:, cols]),
            "wv": pack(wv_aug),
            "wvl": np.ascontiguousarray(
                np.broadcast_to(wvl, (128, VT))).astype(bfloat16),
            "bqk": np.ascontiguousarray(bqk_c.astype(np.float32)),
            "tri": tri,
        }
        for q in range(4):
            m[f"xq{q}"] = xqs[b][q]
        in_maps.append(m)
    return in_maps


def kernel(hidden_states, Wqk, bqk, Wv, bv):
    import time

    from concourse.bass_utils import run_bass_kernel_spmd

    in_maps = make_in_maps(hidden_states, Wqk, bqk, Wv, bv)
    res = None
    for attempt in range(3):
        try:
            res = run_bass_kernel_spmd(_get_nc(), in_maps, list(range(8)))
            break
        except Exception:
            # transient NRT_EXEC_UNIT_UNRECOVERABLE errors have been observed
            # on this fabric; back off and retry
            if attempt == 2:
                raise
            time.sleep(2.0)
    outp = np.empty((B, S, NH * HD), np.float32)
    for c in range(8):
        b, ho = c // 4, (c % 4) * NHL
        # out[r, ((a*2+ht)*2+hh)*512 + q]: head dim r (or denom at r=64)
        # of head 2*ht+hh, query a*512+q
        arr = res.results[c]["out"].astype(np.float32).reshape(
            HD + 1, NA, 2, 2, 512)
        blk = arr[:HD] / arr[HD]                      # [r, a, ht, hh, q]
        outp[b, :, ho * HD:(ho + NHL) * HD] = (
            blk.transpose(1, 4, 2, 3, 0).reshape(S, WC))
    return outp
